# revision 17
# baseline (speedup 1.0000x reference)
# Trainium2 Bass kernel for single-head causal attention
#   q = x@Wq, k = x@Wk, v = x@Wv   (x [B,T,C], W* [C,H])
#   out = softmax(mask(q k^T / sqrt(C))) @ v
# B=512, T=142, C=512, H=64.  Data-parallel over B across 8 NeuronCores.
#
# Strategy (per core, 64 batches = 9088 tokens):
#  - split-fp8 projections: x and 32*W are each split into hi+lo e4m3
#    planes; every projection is 3 DoubleRow terms (xh*Wh + xl*Wh + xh*Wl),
#    contracting 256 rows per pass at 0.5 PE-cycles/column.
#  - QK pass: [Wq|Wk] packed stationary -> psum [128, tokens] (q rows 0:64,
#    k rows 64:128); copied to SBUF bf16. The q half is then shifted to
#    partitions 64:128 via SBUF->SBUF DMA so the scores matmul has both
#    operands at base partition 64 (HW requires equal base partitions).
#  - V pass: x-stationary per batch -> v in natural [token, 64] layout
#    (no transpose), plus a ones column for the softmax denominator.
#  - scores: kT-stationary matmuls + causal mask added via an
#    identity-stationary matmul into PSUM; exp on ScalarE (scale fused).
#  - AV with out = [65 features, tokens]: 156 PE cycles/batch; row 64 is
#    the denominator. Division (and /32 weight-scale) happens on host.
#  - software-pipelined per group (3 batches): input DMA, QK proj, q-shift,
#    V proj of chunk c overlap attention of chunk c-1.  Both groups of a
#    chunk run scores before either runs AV so the exp latency is hidden.
#  - PE warm-up matmuls cover the initial input-DMA latency and the
#    Tensor-engine p-state ramp.
import os

import numpy as np
import ml_dtypes

B, T, C, H = 512, 142, 512, 64
NCORES = 8
NB = B // NCORES            # 64 batches per core
NT = NB * T                 # 9088 tokens per core
GB = 3                      # batches per group
NG = (NB + GB - 1) // GB    # 22 groups (21 full + 1 single)
GT = GB * T                 # 426
XW = 432                    # x tile token stride (16B-aligned >= GT)
WS = 32.0                   # weight pre-scale (power of two: exact in bf16)
EXP_SCALE = float(C) ** -0.5 / (WS * WS)
NEG = -1e30
N_WARMUP = 26

# projection terms as (w_plane, x_plane); 0 = hi, 1 = lo
QK_TERMS = [(0, 0), (0, 1), (1, 0)]
V_TERMS = [(0, 0), (0, 1), (1, 0)]

_CACHE = {}


def _groups():
    return [(g * GB, min(GB, NB - g * GB)) for g in range(NG)]


def _chunks():
    # 10 chunks of 2 full groups, then the last full group and the
    # 1-batch tail group alone (keeps the pipeline tail short)
    return [(2 * c, 2) for c in range(10)] + [(20, 1), (21, 1)]


def _build_nc():
    import concourse.bacc as bacc
    import concourse.mybir as mybir
    from concourse.tile import TileContext

    fp32 = mybir.dt.float32
    bf16 = mybir.dt.bfloat16
    f8e4 = mybir.dt.float8e4
    Exp = mybir.ActivationFunctionType.Exp
    DR = mybir.MatmulPerfMode.DoubleRow

    nc = bacc.Bacc(
        "TRN2",
        target_bir_lowering=False,
        debug=False,
        enable_asserts=False,
        num_devices=NCORES,
    )

    xt8 = nc.dram_tensor("xt8", [NG, 4, 128, 2, XW], f8e4,
                     kind="ExternalInput").ap()
    wqk = nc.dram_tensor("wqk", [128, 8, 128], f8e4, kind="ExternalInput").ap()
    wv8 = nc.dram_tensor("wv8", [128, 8, 64], f8e4, kind="ExternalInput").ap()
    # cst: cols 0:128 causal mask, 128:256 identity128, 256:270 mask14 (rows 0:14)
    cst = nc.dram_tensor("cst", [128, 270], bf16, kind="ExternalInput").ap()
    om = nc.dram_tensor("om", [NG, 65, GT], bf16, kind="ExternalOutput").ap()

    groups = _groups()
    chunks = _chunks()

    with TileContext(nc) as tc:
        with (
            tc.tile_pool(name="const", bufs=1) as cpool,
            tc.tile_pool(name="xin", bufs=8) as xpool,
            tc.tile_pool(name="work", bufs=2) as wpool,
            tc.tile_pool(name="psum", bufs=1, space="PSUM") as ppool,
        ):
            w_sb = cpool.tile([128, 8, 128], f8e4)
            wv_sb = cpool.tile([128, 8, 64], f8e4)
            cst_sb = cpool.tile([128, 270], bf16)
            wsrc = cpool.tile([128, 128], bf16)
            nc.vector.memset(wsrc, 0.125)

            iden = cst_sb[:, 128:256]
            mask = cst_sb[:, 0:128]
            iden14 = cst_sb[0:14, 128:142]
            mskt = cst_sb[0:14, 256:270]

            # ---- PE warm-up: busy until the first x chunk lands, and past
            # the p-state ramp so real matmuls run at full clock ----
            wpo = ppool.tile([65, GT], fp32, tag="po", bufs=2)
            for _ in range(N_WARMUP):
                nc.tensor.matmul(
                    wpo[:, 0:128], lhsT=wsrc[:, 0:65],
                    rhs=wsrc[:, :], start=True, stop=True,
                )

            def dma_x(ci):
                g0, ngrp = chunks[ci]
                tiles = []
                for g in range(g0, g0 + ngrp):
                    xt_t = xpool.tile([128, 4, 2, XW], f8e4, tag="x")
                    nc.sync.dma_start(
                        out=xt_t,
                        in_=xt8[g].rearrange("c p a t -> p c a t"),
                    )
                    tiles.append(xt_t)
                return tiles

            gstate = {}
            cstate = {}

            def attn_scores(ci, g):
                b0, nb, vex = gstate[g]
                qkc, qsc = cstate[ci]
                gc0 = (b0 - groups[chunks[ci][0]][0]) * T
                psc = ppool.tile([128, 468], fp32, tag="psc", bufs=2)
                for j in range(nb):
                    cl = j * T
                    gj = gc0 + cl
                    nc.tensor.matmul(
                        psc[:, cl:cl + 128],
                        lhsT=qkc[64:128, gj:gj + 128],
                        rhs=qsc[64:128, gj:gj + 128],
                        start=True, stop=False,
                    )
                    nc.tensor.matmul(
                        psc[:, cl:cl + 128], lhsT=iden, rhs=mask,
                        start=False, stop=True,
                    )
                    nc.tensor.matmul(
                        psc[:, cl + 128:cl + T],
                        lhsT=qkc[64:128, gj:gj + 128],
                        rhs=qsc[64:128, gj + 128:gj + T],
                        start=True, stop=True,
                    )
                    tc0 = nb * T + j * 14
                    nc.tensor.matmul(
                        psc[0:14, tc0:tc0 + 14],
                        lhsT=qkc[64:128, gj + 128:gj + T],
                        rhs=qsc[64:128, gj + 128:gj + T],
                        start=True, stop=False,
                    )
                    nc.tensor.matmul(
                        psc[0:14, tc0:tc0 + 14], lhsT=iden14, rhs=mskt,
                        start=False, stop=True,
                    )
                exp_t = wpool.tile([128, 468], bf16, tag="exp", bufs=2)
                nc.scalar.activation(
                    exp_t[:, 0:nb * 156], psc[:, 0:nb * 156], Exp,
                    scale=EXP_SCALE,
                )
                gstate[g] = (b0, nb, vex, exp_t)

            def attn_av(g):
                b0, nb, vex, exp_t = gstate.pop(g)
                po = ppool.tile([65, GT], fp32, tag="po", bufs=2)
                for j in range(nb):
                    cl = j * T
                    nc.tensor.matmul(
                        po[:, cl:cl + 128],
                        lhsT=vex[:, j, :], rhs=exp_t[:, cl:cl + 128],
                        start=True, stop=True,
                    )
                    nc.tensor.matmul(
                        po[:, cl + 128:cl + T],
                        lhsT=vex[:, j, :], rhs=exp_t[:, cl + 128:cl + T],
                        start=True, stop=False,
                    )
                    nc.tensor.matmul(
                        po[:, cl + 128:cl + T],
                        lhsT=vex[0:14, GB + j, :],
                        rhs=exp_t[0:14, nb * T + j * 14:nb * T + (j + 1) * 14],
                        start=False, stop=True,
                    )
                return po, nb

            def out_flush(ci, pend):
                g0, ngrp = chunks[ci]
                osb = wpool.tile([65, 2, GT], bf16, tag="osb", bufs=2)
                for s, (po, nb) in enumerate(pend):
                    hl = (nb * T) // 2
                    nc.vector.tensor_copy(osb[:, s, 0:hl], po[:, 0:hl])
                    nc.scalar.copy(osb[:, s, hl:nb * T], po[:, hl:nb * T])
                nc.sync.dma_start(
                    out=om[g0:g0 + ngrp].rearrange("g p c -> p g c"),
                    in_=osb[:, 0:ngrp, :],
                )

            def qk_proj(ci, xts):
                gc0_idx, ngrp = chunks[ci]
                cgroups = groups[gc0_idx:gc0_idx + ngrp]
                t0 = cgroups[0][0] * T
                tlen = sum(nb for _, nb in cgroups) * T
                qkc = wpool.tile([128, 852], bf16, tag="qkc", bufs=6)
                qsc = wpool.tile([128, 852], bf16, tag="qsc", bufs=6)
                cstate[ci] = (qkc, qsc)
                for gi, (b0, nb) in enumerate(cgroups):
                    g = gc0_idx + gi
                    gt = nb * T
                    xt_t = xts[gi]
                    pq = ppool.tile([128, GT], fp32, tag="pq", bufs=2)
                    nmm = len(QK_TERMS) * 2
                    i = 0
                    for (wp, xp) in QK_TERMS:
                        for kp in range(2):
                            wi = wp * 4 + kp * 2
                            nc.tensor.matmul(
                                pq[:, 0:gt],
                                lhsT=w_sb[:, wi:wi + 2, :],
                                rhs=xt_t[:, 2 * kp:2 * kp + 2, xp, 0:gt],
                                start=(i == 0), stop=(i == nmm - 1),
                                perf_mode=DR,
                            )
                            i += 1
                    gcol = (b0 - cgroups[0][0]) * T
                    if g % 2 == 0:
                        nc.vector.tensor_copy(
                            qkc[:, gcol:gcol + gt], pq[:, 0:gt])
                    else:
                        nc.scalar.copy(
                            qkc[:, gcol:gcol + gt], pq[:, 0:gt])
                # q shift to partitions 64:128 (SBUF->SBUF DMA)
                nc.scalar.dma_start(
                    out=qsc[64:128, 0:tlen],
                    in_=qkc[0:64, 0:tlen],
                )

            def v_proj(ci, xts):
                gc0_idx, ngrp = chunks[ci]
                cgroups = groups[gc0_idx:gc0_idx + ngrp]
                for gi, (b0, nb) in enumerate(cgroups):
                    g = gc0_idx + gi
                    xt_t = xts[gi]
                    vm = ppool.tile([128, 384], fp32, tag="vm", bufs=2)
                    nmm = len(V_TERMS) * 2
                    for j in range(nb):
                        tb = j * T
                        i = 0
                        for (wp, xp) in V_TERMS:
                            for kp in range(2):
                                wi = wp * 4 + kp * 2
                                nc.tensor.matmul(
                                    vm[:, j * 64:(j + 1) * 64],
                                    lhsT=xt_t[:, 2 * kp:2 * kp + 2, xp,
                                              tb:tb + 128],
                                    rhs=wv_sb[:, wi:wi + 2, :],
                                    start=(i == 0), stop=(i == nmm - 1),
                                    perf_mode=DR,
                                )
                                i += 1
                        i = 0
                        for (wp, xp) in V_TERMS:
                            for kp in range(2):
                                wi = wp * 4 + kp * 2
                                nc.tensor.matmul(
                                    vm[0:14, 192 + j * 64:192 + (j + 1) * 64],
                                    lhsT=xt_t[:, 2 * kp:2 * kp + 2, xp,
                                              tb + 128:tb + T],
                                    rhs=wv_sb[:, wi:wi + 2, :],
                                    start=(i == 0), stop=(i == nmm - 1),
                                    perf_mode=DR,
                                )
                                i += 1
                    # vex slots 0:3 = v main [128 tok, 64]; slots 3:6 = v tail
                    # (rows 0:14 valid; rows 14:128 stale psum, never read)
                    vex = wpool.tile([128, 2 * GB, 65], bf16, tag="vex",
                                     bufs=8)
                    if nb == GB:
                        nc.vector.tensor_copy(
                            vex[:, 0:6, 0:64],
                            vm[:, 0:384].rearrange("p (b h) -> p b h", h=64),
                        )
                    else:
                        nc.vector.tensor_copy(
                            vex[:, 0:nb, 0:64],
                            vm[:, 0:nb * 64].rearrange(
                                "p (b h) -> p b h", h=64),
                        )
                        nc.vector.tensor_copy(
                            vex[0:14, GB:GB + nb, 0:64],
                            vm[0:14, 192:192 + nb * 64].rearrange(
                                "p (b h) -> p b h", h=64),
                        )
                    nc.gpsimd.memset(vex[:, 0:nb, 64:65], 1.0)
                    nc.gpsimd.memset(vex[0:14, GB:GB + nb, 64:65], 1.0)
                    gstate[g] = (b0, nb, vex)

            # ---- software-pipelined main loop ----
            # iter ci: scores(ci-1) | qk(ci) | AV(ci-1)+out | v(ci) | x(ci+1)
            nc.sync.dma_start(out=w_sb, in_=wqk)
            nc.sync.dma_start(out=wv_sb, in_=wv8)
            LAG = 2
            xq = [dma_x(0)]
            nc.sync.dma_start(out=cst_sb, in_=cst)
            xq.append(dma_x(1))
            for ci in range(len(chunks)):
                if ci + 2 < len(chunks):
                    xq.append(dma_x(ci + 2))
                xts_cur = xq.pop(0)
                if ci >= LAG:
                    pg0, png = chunks[ci - LAG]
                    for g in range(pg0, pg0 + png):
                        attn_scores(ci - LAG, g)
                qk_proj(ci, xts_cur)
                if ci >= LAG:
                    pend = []
                    for g in range(pg0, pg0 + png):
                        pend.append(attn_av(g))
                    out_flush(ci - LAG, pend)
                v_proj(ci, xts_cur)

            # epilogue: attention for the last LAG chunks, pipelined
            # pairwise (max 2 live psc/po tiles)
            rem = []
            for ci in range(len(chunks) - LAG, len(chunks)):
                pg0, png = chunks[ci]
                for g in range(pg0, pg0 + png):
                    rem.append((ci, g))
            flushed = {}
            for idx, (ci, g) in enumerate(rem):
                if idx < 2:
                    attn_scores(ci, g)
                else:
                    ci2, g2 = rem[idx - 2]
                    flushed.setdefault(ci2, []).append(attn_av(g2))
                    if len(flushed[ci2]) == chunks[ci2][1]:
                        out_flush(ci2, flushed.pop(ci2))
                    attn_scores(ci, g)
            for ci2, g2 in rem[-2:]:
                flushed.setdefault(ci2, []).append(attn_av(g2))
                if len(flushed[ci2]) == chunks[ci2][1]:
                    out_flush(ci2, flushed.pop(ci2))

    nc.compile()
    return nc


def _prep_shared(Wq, Wk, Wv):
    f8 = ml_dtypes.float8_e4m3fn
    bf = ml_dtypes.bfloat16

    def split(W):
        Wp = (W * WS).astype(np.float32)
        hi = Wp.astype(f8)
        lo = (Wp - hi.astype(np.float32)).astype(f8)
        return hi, lo

    qk = np.concatenate([Wq, Wk], axis=1)            # [512, 128]
    qk_hi, qk_lo = split(qk)
    v_hi, v_lo = split(Wv)

    def pack(hi, lo, m):
        # [128, 8, m]: index = plane*4 + chunk (chunk = 2*kp + sub)
        a = np.concatenate(
            [hi.reshape(4, 128, m), lo.reshape(4, 128, m)], axis=0)
        return np.ascontiguousarray(a.transpose(1, 0, 2))

    s = np.arange(128)[:, None]
    t = np.arange(128)[None, :]
    msk = np.where(s <= t, 0.0, NEG).astype(np.float32)
    i14 = np.arange(14)[:, None]
    j14 = np.arange(14)[None, :]
    mskt = np.where(i14 <= j14, 0.0, NEG).astype(np.float32)
    cst = np.zeros((128, 270), np.float32)
    cst[:, 0:128] = msk
    cst[:, 128:256] = np.eye(128, dtype=np.float32)
    cst[0:14, 256:270] = mskt

    return dict(
        wqk=pack(qk_hi, qk_lo, 128),
        wv8=pack(v_hi, v_lo, 64),
        cst=cst.astype(bf),
    )


def _prep_core_x(x_core):
    # x_core [NB, T, C] fp32 -> [NG, 4, 128, 2, XW] e4m3: per-group blocks
    # with hi/lo planes interleaved per partition (864B contiguous runs)
    f8 = ml_dtypes.float8_e4m3fn
    xt = np.ascontiguousarray(x_core.reshape(NT, C).T)   # [C, NT]
    hi = xt.astype(f8)
    lo = (xt - hi.astype(np.float32)).astype(f8)
    planes = np.stack([hi.reshape(4, 128, NT), lo.reshape(4, 128, NT)],
                      axis=3)                            # [4, 128, NT, 2]
    out = np.zeros((NG, 4, 128, 2, XW), f8)
    for g, (b0, nb) in enumerate(_groups()):
        blk = planes[:, :, b0 * T:(b0 + nb) * T]         # [4, 128, gt, 2]
        out[g, :, :, :, 0:nb * T] = blk.transpose(0, 1, 3, 2)
    return out


def _assemble_core(om_np):
    # om [NG, 65, GT] bf16 -> [NB, T, H]
    om = om_np.astype(np.float32)
    out = np.empty((NB, T, H), np.float32)
    for g, (b0, nb) in enumerate(_groups()):
        for j in range(nb):
            blk = om[g, :, j * T:(j + 1) * T]       # [65, T]
            out[b0 + j] = (blk[0:64] / blk[64:65]).T / WS
    return out


def kernel(**inputs):
    x = np.asarray(inputs["x"], dtype=np.float32)
    Wq = np.asarray(inputs["Wq"], dtype=np.float32)
    Wk = np.asarray(inputs["Wk"], dtype=np.float32)
    Wv = np.asarray(inputs["Wv"], dtype=np.float32)

    from concourse.bass_utils import run_bass_kernel_spmd

    if "nc" not in _CACHE:
        _CACHE["nc"] = _build_nc()
    nc = _CACHE["nc"]

    shared = _prep_shared(Wq, Wk, Wv)
    in_maps = []
    for core in range(NCORES):
        m = dict(shared)
        m["xt8"] = _prep_core_x(x[core * NB:(core + 1) * NB])
        in_maps.append(m)

    trace = bool(int(os.environ.get("TRN_KERNEL_TRACE", "0")))
    res = run_bass_kernel_spmd(
        nc, in_maps, core_ids=list(range(NCORES)), trace=trace,
    )
    _CACHE["last_result"] = res

    outs = []
    for core in range(NCORES):
        outs.append(_assemble_core(np.asarray(res.results[core]["om"])))
    return np.concatenate(outs, axis=0).astype(np.float32)
